# revision 8
# baseline (speedup 1.0000x reference)
"""Multi-head causal self-attention on 8 Trainium2 NeuronCores.

Sharding: core c -> batch b = c // 2, heads 4*(c % 2) .. +4  (data parallel on
B, tensor parallel on heads).  Each core computes its 4 heads' attention for
its batch plus the partial out-projection; the host sums the two partials per
batch and adds b_out.

Per-core layout (engine-op APs stay at base partition 0; only PE operand
reads use base-64 slices):
  xT   [D+1, T]    x[b] transposed on host + ones row (folds b_qkv in as K+1)
  qT/kT [128, 2, T] head-major: partitions = 2 heads x 64, 2 m-tiles
  v    [128, 16, 260] natural [T, hd] per head + a ones column (gives the
                    softmax denominator for free during the AV matmul)
  scores are computed transposed: sT[k, q] = kT.T @ q, exp'd on ACT during
  PSUM->SBUF evacuation (no max subtraction: |scores| <= ~3 here), causality
  via tile skipping/trimming + one upper-triangular 128x128 mask constant.

Matmuls run in float32r (1 row/cycle vs 4 for float32, ~1.6e-4 rel err).
"""

import os
import sys
from contextlib import ExitStack

import numpy as np

for _p in ("/opt/trn_rl_repo", "/opt/pypackages"):
    if os.path.isdir(_p) and _p not in sys.path:
        sys.path.append(_p)

import concourse.bass as bass
from concourse import bacc
import concourse.mybir as mybir
import concourse.tile as tile
from concourse.bass_utils import run_bass_kernel_spmd
from concourse.masks import make_upper_triangular

B, T, D = 4, 2048, 512
H, HD = 8, 64
HPC = 4  # heads per core
P = 128
KT = D // P  # k-tiles over the model dim
QB = 512  # query block (free dim per matmul)
NQB = T // QB
NKT = T // P  # key tiles
VW = HD + 1  # v columns per head incl. the ones column

F32 = mybir.dt.float32
R32 = mybir.dt.float32r
EXP = mybir.ActivationFunctionType.Exp


def build_bass():
    nc = bacc.Bacc()
    xT = nc.declare_dram_parameter("xT", [D + 1, T], R32, isOutput=False)
    wqa = nc.declare_dram_parameter("wqa", [D + 1, 2 * P], R32, isOutput=False)
    wka = nc.declare_dram_parameter("wka", [D + 1, 2 * P], R32, isOutput=False)
    wva = nc.declare_dram_parameter("wva", [D + 1, HPC * VW], R32, isOutput=False)
    wo = nc.declare_dram_parameter("wo", [HPC * HD, D], R32, isOutput=False)
    y = nc.declare_dram_parameter("y", [T, D], F32, isOutput=True)

    with tile.TileContext(nc) as tc, ExitStack() as ctx:
        consts = ctx.enter_context(tc.tile_pool(name="consts", bufs=1))
        qkv = ctx.enter_context(tc.tile_pool(name="qkv", bufs=1))
        attn = ctx.enter_context(tc.tile_pool(name="attn", bufs=1))
        etp = ctx.enter_context(tc.tile_pool(name="etp", bufs=4))
        nrm = ctx.enter_context(tc.tile_pool(name="nrm", bufs=3))
        yevac = ctx.enter_context(tc.tile_pool(name="yevac", bufs=3))
        mmps = ctx.enter_context(tc.tile_pool(name="mmps", bufs=3, space="PSUM"))
        aps = ctx.enter_context(tc.tile_pool(name="aps", bufs=2, space="PSUM"))
        bcps = ctx.enter_context(tc.tile_pool(name="bcps", bufs=2, space="PSUM"))

        # ---- inputs / constants into SBUF
        x_sb = consts.tile([P, KT, T], R32)
        nc.sync.dma_start(
            out=x_sb, in_=xT[0:D, :].rearrange("(kt p) t -> p kt t", p=P)
        )
        ones_t = consts.tile([1, T], R32)
        nc.sync.dma_start(out=ones_t, in_=xT[D : D + 1, :])
        wq_sb = consts.tile([P, KT, 2 * P], R32)
        nc.sync.dma_start(
            out=wq_sb, in_=wqa[0:D, :].rearrange("(kt p) m -> p kt m", p=P)
        )
        wqb_sb = consts.tile([1, 2 * P], R32)
        nc.sync.dma_start(out=wqb_sb, in_=wqa[D : D + 1, :])
        wk_sb = consts.tile([P, KT, 2 * P], R32)
        nc.sync.dma_start(
            out=wk_sb, in_=wka[0:D, :].rearrange("(kt p) m -> p kt m", p=P)
        )
        wkb_sb = consts.tile([1, 2 * P], R32)
        nc.sync.dma_start(out=wkb_sb, in_=wka[D : D + 1, :])
        wv_sb = consts.tile([P, KT, HPC * VW], R32)
        nc.sync.dma_start(
            out=wv_sb, in_=wva[0:D, :].rearrange("(kt p) m -> p kt m", p=P)
        )
        wvb_sb = consts.tile([1, HPC * VW], R32)
        nc.sync.dma_start(out=wvb_sb, in_=wva[D : D + 1, :])
        wo_sb = consts.tile([HD, HPC, D], R32)
        nc.sync.dma_start(out=wo_sb, in_=wo.rearrange("(h p) d -> p h d", p=HD))

        # triu[k, q] = 1 iff q >= k: allowed region of a diagonal block in
        # transposed-score space.  gpsimd affine_select needs f32; cast after.
        triu_st = consts.tile([P, P], F32)
        make_upper_triangular(nc, triu_st, val=1.0, diag=True)
        triu = consts.tile([P, P], R32)
        nc.vector.tensor_copy(triu, triu_st)
        # lhsT of the denominator-broadcast matmul, row HD (must share base
        # partition with the reciprocal row it multiplies).
        ones_st = consts.tile([P, HD], F32)
        nc.vector.memset(ones_st, 1.0)
        ones_bc = consts.tile([P, HD], R32)
        nc.vector.tensor_copy(ones_bc, ones_st)

        # ---- QKV projections (bias folded in via the ones row, K = D + 1)
        qT_sb = qkv.tile([P, 2, T], R32)
        kT_sb = qkv.tile([P, 2, T], R32)
        v_sb = qkv.tile([P, NKT, HPC * VW], R32)

        for w_sb, wb_sb, dst in ((wq_sb, wqb_sb, qT_sb), (wk_sb, wkb_sb, kT_sb)):
            for m in range(2):
                for nb in range(NQB):
                    ps = mmps.tile([P, QB], F32, tag="mm", name="ps")
                    for kt in range(KT):
                        nc.tensor.matmul(
                            ps,
                            lhsT=w_sb[:, kt, m * P : (m + 1) * P],
                            rhs=x_sb[:, kt, nb * QB : (nb + 1) * QB],
                            start=(kt == 0),
                            stop=False,
                        )
                    nc.tensor.matmul(
                        ps,
                        lhsT=wb_sb[:, m * P : (m + 1) * P],
                        rhs=ones_t[:, nb * QB : (nb + 1) * QB],
                        start=False,
                        stop=True,
                    )
                    nc.scalar.copy(out=dst[:, m, nb * QB : (nb + 1) * QB], in_=ps)

        for tt in range(NKT):
            ps = mmps.tile([P, QB], F32, tag="mm", name="ps")
            for kt in range(KT):
                nc.tensor.matmul(
                    ps[:, 0 : HPC * VW],
                    lhsT=x_sb[:, kt, tt * P : (tt + 1) * P],
                    rhs=wv_sb[:, kt, :],
                    start=(kt == 0),
                    stop=False,
                )
            nc.tensor.matmul(
                ps[:, 0 : HPC * VW],
                lhsT=ones_t[:, tt * P : (tt + 1) * P],
                rhs=wvb_sb,
                start=False,
                stop=True,
            )
            nc.vector.tensor_copy(v_sb[:, tt, :], ps[:, 0 : HPC * VW])

        # ---- attention, per head
        attn_h = [
            attn.tile([HD, T], R32, tag=f"attn{h}", name=f"attn{h}")
            for h in range(HPC)
        ]

        for h in range(HPC):
            qh = qT_sb[(h % 2) * HD : (h % 2) * HD + HD, h // 2, :]
            kh = kT_sb[(h % 2) * HD : (h % 2) * HD + HD, h // 2, :]
            for qb in range(NQB):
                ops = aps.tile([VW, QB], F32, tag="ops", name="ops")
                nkt = (qb + 1) * (QB // P)
                for kt in range(nkt):
                    off = max(0, kt * P - qb * QB)
                    sps = mmps.tile([P, QB], F32, tag="mm", name="sps")
                    nc.tensor.matmul(
                        sps[:, off:QB],
                        lhsT=kh[:, kt * P : (kt + 1) * P],
                        rhs=qh[:, qb * QB + off : (qb + 1) * QB],
                        start=True,
                        stop=True,
                    )
                    eT = etp.tile([P, QB], R32, tag="eT", name="eT")
                    nc.scalar.activation(
                        out=eT[:, off:QB], in_=sps[:, off:QB], func=EXP,
                        scale=1.0 / np.sqrt(HD),
                    )
                    if kt >= qb * (QB // P):  # diagonal-crossing key tile
                        nc.vector.tensor_mul(
                            eT[:, off : off + P], eT[:, off : off + P], triu
                        )
                    nc.tensor.matmul(
                        ops[:, off:QB],
                        lhsT=v_sb[:, kt, h * VW : (h + 1) * VW],
                        rhs=eT[:, off:QB],
                        start=(kt == 0),
                        stop=(kt == nkt - 1),
                    )
                # normalize: row HD of ops is the softmax denominator.
                # (engine ops keep base partition 0: full-tile copy + recip)
                att_sb = nrm.tile([VW, QB], F32, tag="att", name="att_sb")
                nc.scalar.copy(out=att_sb, in_=ops)
                rec = nrm.tile([VW, QB], F32, tag="rec", name="rec")
                nc.vector.reciprocal_approx_fast(out=rec, in_=att_sb)
                recr = nrm.tile([VW, QB], R32, tag="recr", name="recr")
                nc.vector.tensor_copy(recr, rec)
                bc = bcps.tile([HD, QB], F32, tag="bc", name="bc")
                nc.tensor.matmul(
                    bc,
                    lhsT=ones_bc[HD : HD + 1, :],
                    rhs=recr[HD : HD + 1, :],
                    start=True,
                    stop=True,
                )
                nc.vector.tensor_mul(
                    attn_h[h][:, qb * QB : (qb + 1) * QB], att_sb[0:HD, :], bc
                )

        # ---- out projection: y[t, d] = sum_h attn_h.T @ wo_h
        for tt in range(NKT):
            ps = mmps.tile([P, D], F32, tag="mm", name="ps")
            for h in range(HPC):
                nc.tensor.matmul(
                    ps,
                    lhsT=attn_h[h][:, tt * P : (tt + 1) * P],
                    rhs=wo_sb[:, h, :],
                    start=(h == 0),
                    stop=(h == HPC - 1),
                )
            yt = yevac.tile([P, D], F32, tag="yt", name="yt")
            nc.vector.tensor_copy(yt, ps)
            nc.sync.dma_start(out=y[tt * P : (tt + 1) * P, :], in_=yt)

    nc.compile()
    return nc


def make_in_maps(x, W_qkv, b_qkv, W_out):
    x = np.asarray(x, np.float32)
    W_qkv = np.asarray(W_qkv, np.float32)
    b_qkv = np.asarray(b_qkv, np.float32)
    W_out = np.asarray(W_out, np.float32)
    in_maps = []
    for c in range(2 * B):
        b, g = divmod(c, 2)
        ch = g * HPC * HD
        xTa = np.empty((D + 1, T), np.float32)
        xTa[:D] = x[b].T
        xTa[D] = 1.0
        wqa = np.concatenate(
            [W_qkv[:, ch : ch + 256], b_qkv[None, ch : ch + 256]], axis=0
        )
        wka = np.concatenate(
            [W_qkv[:, D + ch : D + ch + 256], b_qkv[None, D + ch : D + ch + 256]],
            axis=0,
        )
        wva = np.zeros((D + 1, HPC * VW), np.float32)
        wva3 = wva[:D].reshape(D, HPC, VW)
        wva3[:, :, :HD] = W_qkv[:, 2 * D + ch : 2 * D + ch + 256].reshape(D, HPC, HD)
        wvab = wva[D].reshape(HPC, VW)
        wvab[:, :HD] = b_qkv[2 * D + ch : 2 * D + ch + 256].reshape(HPC, HD)
        wvab[:, HD] = 1.0
        in_maps.append(
            {
                "xT": xTa,
                "wqa": np.ascontiguousarray(wqa),
                "wka": np.ascontiguousarray(wka),
                "wva": wva,
                "wo": np.ascontiguousarray(W_out[ch : ch + 256, :]),
            }
        )
    return in_maps


def assemble(results, b_out):
    b_out = np.asarray(b_out, np.float32)
    out = np.empty((B, T, D), np.float32)
    for b in range(B):
        out[b] = results[2 * b]["y"] + results[2 * b + 1]["y"] + b_out[None, :]
    return out


_CACHE = {}


def kernel(x, W_qkv, b_qkv, W_out, b_out):
    if "nc" not in _CACHE:
        _CACHE["nc"] = build_bass()
    in_maps = make_in_maps(x, W_qkv, b_qkv, W_out)
    res = run_bass_kernel_spmd(_CACHE["nc"], in_maps, list(range(2 * B)))
    return assemble(res.results, b_out)


# revision 9
# speedup vs baseline: 1.4863x; 1.4863x over previous
"""Multi-head causal self-attention on 8 Trainium2 NeuronCores.

Sharding: core c -> batch b = c // 2, heads 4*(c % 2) .. +4  (data parallel on
B, tensor parallel on heads).  Each core computes its 4 heads' attention for
its batch plus the partial out-projection; the host sums the two partials per
batch and adds b_out.

Per-core layout (engine-op APs stay at base partition 0; only PE operand
reads use base-64 slices):
  xT   [D+1, T]    x[b] transposed on host + ones row (folds b_qkv in as K+1)
  qT/kT [128, 2, T] head-major: partitions = 2 heads x 64, 2 m-tiles
  v    [128, 16, 260] natural [T, hd] per head + a ones column (gives the
                    softmax denominator for free during the AV matmul)
  scores are computed transposed: sT[k, q] = kT.T @ q, exp'd on ACT during
  PSUM->SBUF evacuation (no max subtraction: |scores| <= ~3 here), causality
  via tile skipping/trimming + one upper-triangular 128x128 mask constant.

Matmuls run in float32r (1 row/cycle vs 4 for float32, ~1.6e-4 rel err).
"""

import os
import sys
from contextlib import ExitStack

import numpy as np

for _p in ("/opt/trn_rl_repo", "/opt/pypackages"):
    if os.path.isdir(_p) and _p not in sys.path:
        sys.path.append(_p)

import concourse.bass as bass
from concourse import bacc
import concourse.mybir as mybir
import concourse.tile as tile
from concourse.bass_utils import run_bass_kernel_spmd
from concourse.masks import make_upper_triangular

B, T, D = 4, 2048, 512
H, HD = 8, 64
HPC = 4  # heads per core
P = 128
KT = D // P  # k-tiles over the model dim
QB = 512  # query block (free dim per matmul)
NQB = T // QB
NKT = T // P  # key tiles
VW = HD + 1  # v columns per head incl. the ones column

F32 = mybir.dt.float32
R32 = mybir.dt.float32r
BF16 = mybir.dt.bfloat16
# Matmul compute dtype for everything except the normalization broadcast
# (which stays f32r so the softmax denominators keep full precision).
MMDT = BF16 if os.environ.get("MHSA_DTYPE", "bf16") == "bf16" else R32
EXP = mybir.ActivationFunctionType.Exp

try:
    import ml_dtypes
    _NP_MMDT = np.float32 if MMDT == R32 else np.dtype(ml_dtypes.bfloat16)
except ImportError:
    _NP_MMDT = np.float32


def build_bass():
    nc = bacc.Bacc()
    xT = nc.declare_dram_parameter("xT", [D + 1, T], MMDT, isOutput=False)
    wqa = nc.declare_dram_parameter("wqa", [D + 1, 2 * P], MMDT, isOutput=False)
    wka = nc.declare_dram_parameter("wka", [D + 1, 2 * P], MMDT, isOutput=False)
    wva = nc.declare_dram_parameter("wva", [D + 1, HPC * VW], MMDT, isOutput=False)
    wo = nc.declare_dram_parameter("wo", [HPC * HD, D], MMDT, isOutput=False)
    y = nc.declare_dram_parameter("y", [T, D], F32, isOutput=True)

    with tile.TileContext(nc) as tc, ExitStack() as ctx:
        consts = ctx.enter_context(tc.tile_pool(name="consts", bufs=1))
        qkv = ctx.enter_context(tc.tile_pool(name="qkv", bufs=1))
        attn = ctx.enter_context(tc.tile_pool(name="attn", bufs=1))
        etp = ctx.enter_context(tc.tile_pool(name="etp", bufs=4))
        nrm = ctx.enter_context(tc.tile_pool(name="nrm", bufs=3))
        yevac = ctx.enter_context(tc.tile_pool(name="yevac", bufs=3))
        mmps = ctx.enter_context(tc.tile_pool(name="mmps", bufs=3, space="PSUM"))
        aps = ctx.enter_context(tc.tile_pool(name="aps", bufs=2, space="PSUM"))
        bcps = ctx.enter_context(tc.tile_pool(name="bcps", bufs=2, space="PSUM"))

        # ---- inputs / constants into SBUF
        x_sb = consts.tile([P, KT, T], MMDT)
        nc.sync.dma_start(
            out=x_sb, in_=xT[0:D, :].rearrange("(kt p) t -> p kt t", p=P)
        )
        ones_t = consts.tile([1, T], MMDT)
        nc.sync.dma_start(out=ones_t, in_=xT[D : D + 1, :])
        wq_sb = consts.tile([P, KT, 2 * P], MMDT)
        nc.sync.dma_start(
            out=wq_sb, in_=wqa[0:D, :].rearrange("(kt p) m -> p kt m", p=P)
        )
        wqb_sb = consts.tile([1, 2 * P], MMDT)
        nc.sync.dma_start(out=wqb_sb, in_=wqa[D : D + 1, :])
        wk_sb = consts.tile([P, KT, 2 * P], MMDT)
        nc.sync.dma_start(
            out=wk_sb, in_=wka[0:D, :].rearrange("(kt p) m -> p kt m", p=P)
        )
        wkb_sb = consts.tile([1, 2 * P], MMDT)
        nc.sync.dma_start(out=wkb_sb, in_=wka[D : D + 1, :])
        wv_sb = consts.tile([P, KT, HPC * VW], MMDT)
        nc.sync.dma_start(
            out=wv_sb, in_=wva[0:D, :].rearrange("(kt p) m -> p kt m", p=P)
        )
        wvb_sb = consts.tile([1, HPC * VW], MMDT)
        nc.sync.dma_start(out=wvb_sb, in_=wva[D : D + 1, :])
        wo_sb = consts.tile([HD, HPC, D], MMDT)
        nc.sync.dma_start(out=wo_sb, in_=wo.rearrange("(h p) d -> p h d", p=HD))

        # triu[k, q] = 1 iff q >= k: allowed region of a diagonal block in
        # transposed-score space.  gpsimd affine_select needs f32; cast after.
        triu_st = consts.tile([P, P], F32)
        make_upper_triangular(nc, triu_st, val=1.0, diag=True)
        triu = consts.tile([P, P], MMDT)
        nc.vector.tensor_copy(triu, triu_st)
        # lhsT of the denominator-broadcast matmul, row HD (must share base
        # partition with the reciprocal row it multiplies).
        ones_st = consts.tile([P, HD], F32)
        nc.vector.memset(ones_st, 1.0)
        ones_bc = consts.tile([P, HD], R32)
        nc.vector.tensor_copy(ones_bc, ones_st)

        # ---- QKV projections (bias folded in via the ones row, K = D + 1)
        qT_sb = qkv.tile([P, 2, T], MMDT)
        kT_sb = qkv.tile([P, 2, T], MMDT)
        v_sb = qkv.tile([P, NKT, HPC * VW], MMDT)

        for w_sb, wb_sb, dst in ((wq_sb, wqb_sb, qT_sb), (wk_sb, wkb_sb, kT_sb)):
            for m in range(2):
                for nb in range(NQB):
                    ps = mmps.tile([P, QB], F32, tag="mm", name="ps")
                    for kt in range(KT):
                        nc.tensor.matmul(
                            ps,
                            lhsT=w_sb[:, kt, m * P : (m + 1) * P],
                            rhs=x_sb[:, kt, nb * QB : (nb + 1) * QB],
                            start=(kt == 0),
                            stop=False,
                        )
                    nc.tensor.matmul(
                        ps,
                        lhsT=wb_sb[:, m * P : (m + 1) * P],
                        rhs=ones_t[:, nb * QB : (nb + 1) * QB],
                        start=False,
                        stop=True,
                    )
                    nc.scalar.copy(out=dst[:, m, nb * QB : (nb + 1) * QB], in_=ps)

        for tt in range(NKT):
            ps = mmps.tile([P, QB], F32, tag="mm", name="ps")
            for kt in range(KT):
                nc.tensor.matmul(
                    ps[:, 0 : HPC * VW],
                    lhsT=x_sb[:, kt, tt * P : (tt + 1) * P],
                    rhs=wv_sb[:, kt, :],
                    start=(kt == 0),
                    stop=False,
                )
            nc.tensor.matmul(
                ps[:, 0 : HPC * VW],
                lhsT=ones_t[:, tt * P : (tt + 1) * P],
                rhs=wvb_sb,
                start=False,
                stop=True,
            )
            nc.vector.tensor_copy(v_sb[:, tt, :], ps[:, 0 : HPC * VW])

        # ---- attention, per head
        attn_h = [
            attn.tile([HD, T], MMDT, tag=f"attn{h}", name=f"attn{h}")
            for h in range(HPC)
        ]

        for h in range(HPC):
            qh = qT_sb[(h % 2) * HD : (h % 2) * HD + HD, h // 2, :]
            kh = kT_sb[(h % 2) * HD : (h % 2) * HD + HD, h // 2, :]
            for qb in range(NQB):
                ops = aps.tile([VW, QB], F32, tag="ops", name="ops")
                nkt = (qb + 1) * (QB // P)
                for kt in range(nkt):
                    off = max(0, kt * P - qb * QB)
                    sps = mmps.tile([P, QB], F32, tag="mm", name="sps")
                    nc.tensor.matmul(
                        sps[:, off:QB],
                        lhsT=kh[:, kt * P : (kt + 1) * P],
                        rhs=qh[:, qb * QB + off : (qb + 1) * QB],
                        start=True,
                        stop=True,
                    )
                    eT = etp.tile([P, QB], MMDT, tag="eT", name="eT")
                    nc.scalar.activation(
                        out=eT[:, off:QB], in_=sps[:, off:QB], func=EXP,
                        scale=1.0 / np.sqrt(HD),
                    )
                    if kt >= qb * (QB // P):  # diagonal-crossing key tile
                        nc.vector.tensor_mul(
                            eT[:, off : off + P], eT[:, off : off + P], triu
                        )
                    nc.tensor.matmul(
                        ops[:, off:QB],
                        lhsT=v_sb[:, kt, h * VW : (h + 1) * VW],
                        rhs=eT[:, off:QB],
                        start=(kt == 0),
                        stop=(kt == nkt - 1),
                    )
                # normalize: row HD of ops is the softmax denominator.
                # (engine ops keep base partition 0: full-tile copy + recip)
                att_sb = nrm.tile([VW, QB], F32, tag="att", name="att_sb")
                nc.scalar.copy(out=att_sb, in_=ops)
                rec = nrm.tile([VW, QB], F32, tag="rec", name="rec")
                nc.vector.reciprocal_approx_fast(out=rec, in_=att_sb)
                recr = nrm.tile([VW, QB], R32, tag="recr", name="recr")
                nc.vector.tensor_copy(recr, rec)
                bc = bcps.tile([HD, QB], F32, tag="bc", name="bc")
                nc.tensor.matmul(
                    bc,
                    lhsT=ones_bc[HD : HD + 1, :],
                    rhs=recr[HD : HD + 1, :],
                    start=True,
                    stop=True,
                )
                nc.vector.tensor_mul(
                    attn_h[h][:, qb * QB : (qb + 1) * QB], att_sb[0:HD, :], bc
                )

        # ---- out projection: y[t, d] = sum_h attn_h.T @ wo_h
        for tt in range(NKT):
            ps = mmps.tile([P, D], F32, tag="mm", name="ps")
            for h in range(HPC):
                nc.tensor.matmul(
                    ps,
                    lhsT=attn_h[h][:, tt * P : (tt + 1) * P],
                    rhs=wo_sb[:, h, :],
                    start=(h == 0),
                    stop=(h == HPC - 1),
                )
            yt = yevac.tile([P, D], F32, tag="yt", name="yt")
            nc.vector.tensor_copy(yt, ps)
            nc.sync.dma_start(out=y[tt * P : (tt + 1) * P, :], in_=yt)

    nc.compile()
    return nc


def make_in_maps(x, W_qkv, b_qkv, W_out):
    x = np.asarray(x, np.float32)
    W_qkv = np.asarray(W_qkv, np.float32)
    b_qkv = np.asarray(b_qkv, np.float32)
    W_out = np.asarray(W_out, np.float32)
    in_maps = []
    for c in range(2 * B):
        b, g = divmod(c, 2)
        ch = g * HPC * HD
        xTa = np.empty((D + 1, T), np.float32)
        xTa[:D] = x[b].T
        xTa[D] = 1.0
        wqa = np.concatenate(
            [W_qkv[:, ch : ch + 256], b_qkv[None, ch : ch + 256]], axis=0
        )
        wka = np.concatenate(
            [W_qkv[:, D + ch : D + ch + 256], b_qkv[None, D + ch : D + ch + 256]],
            axis=0,
        )
        wva = np.zeros((D + 1, HPC * VW), np.float32)
        wva3 = wva[:D].reshape(D, HPC, VW)
        wva3[:, :, :HD] = W_qkv[:, 2 * D + ch : 2 * D + ch + 256].reshape(D, HPC, HD)
        wvab = wva[D].reshape(HPC, VW)
        wvab[:, :HD] = b_qkv[2 * D + ch : 2 * D + ch + 256].reshape(HPC, HD)
        wvab[:, HD] = 1.0
        in_maps.append(
            {
                "xT": xTa.astype(_NP_MMDT),
                "wqa": np.ascontiguousarray(wqa).astype(_NP_MMDT),
                "wka": np.ascontiguousarray(wka).astype(_NP_MMDT),
                "wva": wva.astype(_NP_MMDT),
                "wo": np.ascontiguousarray(W_out[ch : ch + 256, :]).astype(_NP_MMDT),
            }
        )
    return in_maps


def assemble(results, b_out):
    b_out = np.asarray(b_out, np.float32)
    out = np.empty((B, T, D), np.float32)
    for b in range(B):
        out[b] = results[2 * b]["y"] + results[2 * b + 1]["y"] + b_out[None, :]
    return out


_CACHE = {}


def kernel(x, W_qkv, b_qkv, W_out, b_out):
    if "nc" not in _CACHE:
        _CACHE["nc"] = build_bass()
    in_maps = make_in_maps(x, W_qkv, b_qkv, W_out)
    res = run_bass_kernel_spmd(_CACHE["nc"], in_maps, list(range(2 * B)))
    return assemble(res.results, b_out)


# revision 12
# speedup vs baseline: 1.5090x; 1.0153x over previous
"""Multi-head causal self-attention on 8 Trainium2 NeuronCores.

Sharding: core c -> batch b = c // 2, heads 4*(c % 2) .. +4  (data parallel on
B, tensor parallel on heads).  Each core computes its 4 heads' attention for
its batch plus the partial out-projection; the host sums the two partials per
batch and adds b_out.

Per-core layout (engine-op APs stay at base partition 0; only PE operand
reads use base-64 slices):
  xT   [D+1, T]    x[b] transposed on host + ones row (folds b_qkv in as K+1)
  qT/kT [128, 2, T] head-major: partitions = 2 heads x 64, 2 m-tiles
  v    [128, 16, 260] natural [T, hd] per head + a ones column (gives the
                    softmax denominator for free during the AV matmul)
  scores are computed transposed: sT[k, q] = kT.T @ q, exp'd on ACT during
  PSUM->SBUF evacuation (no max subtraction: |scores| <= ~3 here), causality
  via tile skipping/trimming + one upper-triangular 128x128 mask constant.

Matmuls run in float32r (1 row/cycle vs 4 for float32, ~1.6e-4 rel err).
"""

import os
import sys
from contextlib import ExitStack

import numpy as np

for _p in ("/opt/trn_rl_repo", "/opt/pypackages"):
    if os.path.isdir(_p) and _p not in sys.path:
        sys.path.append(_p)

import concourse.bass as bass
from concourse import bacc
import concourse.mybir as mybir
import concourse.tile as tile
from concourse.bass_utils import run_bass_kernel_spmd
from concourse.masks import make_upper_triangular

B, T, D = 4, 2048, 512
H, HD = 8, 64
HPC = 4  # heads per core
P = 128
KT = D // P  # k-tiles over the model dim
QB = 512  # query block (free dim per matmul)
NQB = T // QB
NKT = T // P  # key tiles
VW = HD + 1  # v columns per head incl. the ones column

F32 = mybir.dt.float32
R32 = mybir.dt.float32r
BF16 = mybir.dt.bfloat16
# Matmul compute dtype for everything except the normalization broadcast
# (which stays f32r so the softmax denominators keep full precision).
MMDT = BF16 if os.environ.get("MHSA_DTYPE", "bf16") == "bf16" else R32
EXP = mybir.ActivationFunctionType.Exp

try:
    import ml_dtypes
    _NP_MMDT = np.float32 if MMDT == R32 else np.dtype(ml_dtypes.bfloat16)
except ImportError:
    _NP_MMDT = np.float32


def build_bass():
    nc = bacc.Bacc()
    xT = nc.declare_dram_parameter("xT", [D + 1, T], MMDT, isOutput=False)
    wqa = nc.declare_dram_parameter("wqa", [D + 1, 2 * P], MMDT, isOutput=False)
    wka = nc.declare_dram_parameter("wka", [D + 1, 2 * P], MMDT, isOutput=False)
    wva = nc.declare_dram_parameter("wva", [D + 1, HPC * VW], MMDT, isOutput=False)
    wo = nc.declare_dram_parameter("wo", [HPC * HD, D], MMDT, isOutput=False)
    y = nc.declare_dram_parameter("y", [T, D], F32, isOutput=True)

    with tile.TileContext(nc) as tc, ExitStack() as ctx:
        consts = ctx.enter_context(tc.tile_pool(name="consts", bufs=1))
        qkv = ctx.enter_context(tc.tile_pool(name="qkv", bufs=1))
        attn = ctx.enter_context(tc.tile_pool(name="attn", bufs=1))
        etp = ctx.enter_context(tc.tile_pool(name="etp", bufs=4))
        nrm = ctx.enter_context(tc.tile_pool(name="nrm", bufs=3))
        yevac = ctx.enter_context(tc.tile_pool(name="yevac", bufs=3))
        mmps = ctx.enter_context(tc.tile_pool(name="mmps", bufs=2, space="PSUM"))
        aps = ctx.enter_context(tc.tile_pool(name="aps", bufs=1, space="PSUM"))
        bcps = mmps  # bc tiles share the mm slots

        # ---- inputs / constants into SBUF
        x_sb = consts.tile([P, KT, T], MMDT)
        nc.sync.dma_start(
            out=x_sb, in_=xT[0:D, :].rearrange("(kt p) t -> p kt t", p=P)
        )
        ones_t = consts.tile([1, T], MMDT)
        nc.sync.dma_start(out=ones_t, in_=xT[D : D + 1, :])
        wq_sb = consts.tile([P, KT, 2 * P], MMDT)
        nc.sync.dma_start(
            out=wq_sb, in_=wqa[0:D, :].rearrange("(kt p) m -> p kt m", p=P)
        )
        wqb_sb = consts.tile([1, 2 * P], MMDT)
        nc.sync.dma_start(out=wqb_sb, in_=wqa[D : D + 1, :])
        wk_sb = consts.tile([P, KT, 2 * P], MMDT)
        nc.sync.dma_start(
            out=wk_sb, in_=wka[0:D, :].rearrange("(kt p) m -> p kt m", p=P)
        )
        wkb_sb = consts.tile([1, 2 * P], MMDT)
        nc.sync.dma_start(out=wkb_sb, in_=wka[D : D + 1, :])
        wv_sb = consts.tile([P, KT, HPC * VW], MMDT)
        nc.sync.dma_start(
            out=wv_sb, in_=wva[0:D, :].rearrange("(kt p) m -> p kt m", p=P)
        )
        wvb_sb = consts.tile([1, HPC * VW], MMDT)
        nc.sync.dma_start(out=wvb_sb, in_=wva[D : D + 1, :])
        wo_sb = consts.tile([HD, HPC, D], MMDT)
        nc.sync.dma_start(out=wo_sb, in_=wo.rearrange("(h p) d -> p h d", p=HD))

        # triu[k, q] = 1 iff q >= k: allowed region of a diagonal block in
        # transposed-score space.  gpsimd affine_select needs f32; cast after.
        triu_st = consts.tile([P, P], F32)
        make_upper_triangular(nc, triu_st, val=1.0, diag=True)
        triu = consts.tile([P, P], MMDT)
        nc.vector.tensor_copy(triu, triu_st)
        # lhsT of the denominator-broadcast matmul, row HD (must share base
        # partition with the reciprocal row it multiplies).
        ones_st = consts.tile([P, HD], F32)
        nc.vector.memset(ones_st, 1.0)
        ones_bc = consts.tile([P, HD], R32)
        nc.vector.tensor_copy(ones_bc, ones_st)

        # ---- QKV projections (bias folded in via the ones row, K = D + 1)
        qT_sb = qkv.tile([P, 2, T], MMDT)
        kT_sb = qkv.tile([P, 2, T], MMDT)
        v_sb = qkv.tile([P, NKT, HPC * VW], MMDT)

        WB = 1024  # bf16 moving-operand max
        for w_sb, wb_sb, dst in ((wq_sb, wqb_sb, qT_sb), (wk_sb, wkb_sb, kT_sb)):
            for m in range(2):
                for nb in range(T // WB):
                    ps = mmps.tile([P, WB], F32, tag="mm", name="ps")
                    for lo in range(0, WB, QB):  # psum-bank-sized writes
                        for kt in range(KT):
                            nc.tensor.matmul(
                                ps[:, lo : lo + QB],
                                lhsT=w_sb[:, kt, m * P : (m + 1) * P],
                                rhs=x_sb[:, kt, nb * WB + lo : nb * WB + lo + QB],
                                start=(kt == 0),
                                stop=False,
                            )
                        nc.tensor.matmul(
                            ps[:, lo : lo + QB],
                            lhsT=wb_sb[:, m * P : (m + 1) * P],
                            rhs=ones_t[:, nb * WB + lo : nb * WB + lo + QB],
                            start=False,
                            stop=True,
                        )
                    nc.scalar.copy(out=dst[:, m, nb * WB : (nb + 1) * WB], in_=ps)

        for tt in range(NKT):
            ps = mmps.tile([P, QB], F32, tag="mm", name="ps")
            for kt in range(KT):
                nc.tensor.matmul(
                    ps[:, 0 : HPC * VW],
                    lhsT=x_sb[:, kt, tt * P : (tt + 1) * P],
                    rhs=wv_sb[:, kt, :],
                    start=(kt == 0),
                    stop=False,
                )
            nc.tensor.matmul(
                ps[:, 0 : HPC * VW],
                lhsT=ones_t[:, tt * P : (tt + 1) * P],
                rhs=wvb_sb,
                start=False,
                stop=True,
            )
            nc.vector.tensor_copy(v_sb[:, tt, :], ps[:, 0 : HPC * VW])

        # ---- attention, per head
        attn_h = [
            attn.tile([HD, T], MMDT, tag=f"attn{h}", name=f"attn{h}")
            for h in range(HPC)
        ]

        AB = 1024  # attention query-block width (bf16 moving max)
        NAB = T // AB
        for hp in range(HPC // 2):
            pair = (2 * hp, 2 * hp + 1)
            qhs = [qT_sb[(h % 2) * HD : (h % 2) * HD + HD, h // 2, :] for h in pair]
            khs = [kT_sb[(h % 2) * HD : (h % 2) * HD + HD, h // 2, :] for h in pair]
            for qb in range(NAB):
                opss = [
                    aps.tile([VW, AB], F32, tag=f"ops{i}", name=f"ops{i}")
                    for i in range(2)
                ]
                nkt = (qb + 1) * (AB // P)
                half0_last = min(nkt, qb * (AB // P) + QB // P) - 1
                for kt in range(nkt):
                    off = max(0, kt * P - qb * AB)
                    halves = [
                        (max(off, lo), lo + QB, lo)
                        for lo in range(0, AB, QB)
                        if off < lo + QB
                    ]
                    spss = []
                    for i in (0, 1):  # score matmuls: K=64 row-packed pair
                        sps = mmps.tile([P, AB], F32, tag="mm", name="sps")
                        for o, hi, _lo in halves:
                            nc.tensor.matmul(
                                sps[:, o:hi],
                                lhsT=khs[i][:, kt * P : (kt + 1) * P],
                                rhs=qhs[i][:, qb * AB + o : qb * AB + hi],
                                start=True,
                                stop=True,
                            )
                        spss.append(sps)
                    for i in (0, 1):
                        eT = etp.tile([P, AB], MMDT, tag="eT", name="eT")
                        nc.scalar.activation(
                            out=eT[:, off:AB], in_=spss[i][:, off:AB], func=EXP,
                            scale=1.0 / np.sqrt(HD),
                        )
                        if off > 0 or kt * P == qb * AB:  # diagonal-crossing
                            nc.vector.tensor_mul(
                                eT[:, off : off + P], eT[:, off : off + P], triu
                            )
                        for o, hi, lo in halves:
                            nc.tensor.matmul(
                                opss[i][:, o:hi],
                                lhsT=v_sb[:, kt, pair[i] * VW : (pair[i] + 1) * VW],
                                rhs=eT[:, o:hi],
                                start=(kt == 0),
                                stop=(kt == (half0_last if lo == 0 else nkt - 1)),
                            )
                # normalize: row HD of ops is the softmax denominator.
                # (engine ops keep base partition 0: full-tile copy + recip)
                for i in (0, 1):
                    att_sb = nrm.tile([VW, AB], F32, tag="att", name="att_sb")
                    nc.scalar.copy(out=att_sb, in_=opss[i])
                    rec = nrm.tile([VW, AB], F32, tag="rec", name="rec")
                    nc.vector.reciprocal_approx_fast(out=rec, in_=att_sb)
                    recr = nrm.tile([VW, AB], R32, tag="recr", name="recr")
                    nc.vector.tensor_copy(recr, rec)
                    bc = bcps.tile([HD, AB], F32, tag="mm", name="bc")
                    for j in range(AB // QB):  # f32r moving max is 512
                        nc.tensor.matmul(
                            bc[:, j * QB : (j + 1) * QB],
                            lhsT=ones_bc[HD : HD + 1, :],
                            rhs=recr[HD : HD + 1, j * QB : (j + 1) * QB],
                            start=True,
                            stop=True,
                        )
                    nc.vector.tensor_mul(
                        attn_h[pair[i]][:, qb * AB : (qb + 1) * AB],
                        att_sb[0:HD, :],
                        bc,
                    )

        # ---- out projection: y[t, d] = sum_h attn_h.T @ wo_h
        for tt in range(NKT):
            ps = mmps.tile([P, D], F32, tag="mm", name="ps")
            for h in range(HPC):
                nc.tensor.matmul(
                    ps,
                    lhsT=attn_h[h][:, tt * P : (tt + 1) * P],
                    rhs=wo_sb[:, h, :],
                    start=(h == 0),
                    stop=(h == HPC - 1),
                )
            yt = yevac.tile([P, D], F32, tag="yt", name="yt")
            nc.vector.tensor_copy(yt, ps)
            nc.sync.dma_start(out=y[tt * P : (tt + 1) * P, :], in_=yt)

    nc.compile()
    return nc


def make_in_maps(x, W_qkv, b_qkv, W_out):
    x = np.asarray(x, np.float32)
    W_qkv = np.asarray(W_qkv, np.float32)
    b_qkv = np.asarray(b_qkv, np.float32)
    W_out = np.asarray(W_out, np.float32)
    in_maps = []
    for c in range(2 * B):
        b, g = divmod(c, 2)
        ch = g * HPC * HD
        xTa = np.empty((D + 1, T), np.float32)
        xTa[:D] = x[b].T
        xTa[D] = 1.0
        wqa = np.concatenate(
            [W_qkv[:, ch : ch + 256], b_qkv[None, ch : ch + 256]], axis=0
        )
        wka = np.concatenate(
            [W_qkv[:, D + ch : D + ch + 256], b_qkv[None, D + ch : D + ch + 256]],
            axis=0,
        )
        wva = np.zeros((D + 1, HPC * VW), np.float32)
        wva3 = wva[:D].reshape(D, HPC, VW)
        wva3[:, :, :HD] = W_qkv[:, 2 * D + ch : 2 * D + ch + 256].reshape(D, HPC, HD)
        wvab = wva[D].reshape(HPC, VW)
        wvab[:, :HD] = b_qkv[2 * D + ch : 2 * D + ch + 256].reshape(HPC, HD)
        wvab[:, HD] = 1.0
        in_maps.append(
            {
                "xT": xTa.astype(_NP_MMDT),
                "wqa": np.ascontiguousarray(wqa).astype(_NP_MMDT),
                "wka": np.ascontiguousarray(wka).astype(_NP_MMDT),
                "wva": wva.astype(_NP_MMDT),
                "wo": np.ascontiguousarray(W_out[ch : ch + 256, :]).astype(_NP_MMDT),
            }
        )
    return in_maps


def assemble(results, b_out):
    b_out = np.asarray(b_out, np.float32)
    out = np.empty((B, T, D), np.float32)
    for b in range(B):
        out[b] = results[2 * b]["y"] + results[2 * b + 1]["y"] + b_out[None, :]
    return out


_CACHE = {}


def kernel(x, W_qkv, b_qkv, W_out, b_out):
    if "nc" not in _CACHE:
        _CACHE["nc"] = build_bass()
    in_maps = make_in_maps(x, W_qkv, b_qkv, W_out)
    res = run_bass_kernel_spmd(_CACHE["nc"], in_maps, list(range(2 * B)))
    return assemble(res.results, b_out)


# revision 15
# speedup vs baseline: 1.9529x; 1.2941x over previous
"""Multi-head causal self-attention on 8 Trainium2 NeuronCores.

Sharding: core c -> batch b = c // 2, heads 4*(c % 2) .. +4  (data parallel on
B, tensor parallel on heads).  Each core computes its 4 heads' attention for
its batch plus the partial out-projection; the host sums the two partials per
batch and adds b_out.

Per-core layout (engine-op APs stay at base partition 0; only PE operand
reads use base-64 slices):
  xT   [D+1, T]    x[b] transposed on host + ones row (folds b_qkv in as K+1)
  qT/kT [128, 2, T] head-major: partitions = 2 heads x 64, 2 m-tiles
  v    [128, 16, 260] natural [T, hd] per head + a ones column (gives the
                    softmax denominator for free during the AV matmul)
  scores are computed transposed: sT[k, q] = kT.T @ q, exp'd on ACT during
  PSUM->SBUF evacuation (no max subtraction: |scores| <= ~3 here), causality
  via tile skipping/trimming + one upper-triangular 128x128 mask constant.

Matmuls run in float32r (1 row/cycle vs 4 for float32, ~1.6e-4 rel err).
"""

import os
import sys
from contextlib import ExitStack

import numpy as np

for _p in ("/opt/trn_rl_repo", "/opt/pypackages"):
    if os.path.isdir(_p) and _p not in sys.path:
        sys.path.append(_p)

import concourse.bass as bass
from concourse import bacc
import concourse.mybir as mybir
import concourse.tile as tile
from concourse.bass_utils import run_bass_kernel_spmd
from concourse.masks import make_upper_triangular

B, T, D = 4, 2048, 512
H, HD = 8, 64
HPC = 4  # heads per core
P = 128
KT = D // P  # k-tiles over the model dim
QB = 512  # query block (free dim per matmul)
NQB = T // QB
NKT = T // P  # key tiles
VW = HD + 1  # v columns per head incl. the ones column

F32 = mybir.dt.float32
R32 = mybir.dt.float32r
BF16 = mybir.dt.bfloat16
# Matmul compute dtype for everything except the normalization broadcast
# (which stays f32r so the softmax denominators keep full precision).
MMDT = BF16 if os.environ.get("MHSA_DTYPE", "bf16") == "bf16" else R32
EXP = mybir.ActivationFunctionType.Exp

try:
    import ml_dtypes
    _NP_MMDT = np.float32 if MMDT == R32 else np.dtype(ml_dtypes.bfloat16)
except ImportError:
    _NP_MMDT = np.float32


def build_bass():
    nc = bacc.Bacc()
    xT = nc.declare_dram_parameter("xT", [D + 1, T], MMDT, isOutput=False)
    wqa = nc.declare_dram_parameter("wqa", [D, 2 * P], MMDT, isOutput=False)
    wka = nc.declare_dram_parameter("wka", [D, 2 * P], MMDT, isOutput=False)
    # q/k biases, laid out [channel % 128, channel // 128] for ACT bias APs
    wqkb = nc.declare_dram_parameter("wqkb", [P, 4], F32, isOutput=False)
    wva = nc.declare_dram_parameter("wva", [D + 1, HPC * VW], MMDT, isOutput=False)
    wo = nc.declare_dram_parameter("wo", [HPC * HD, D], MMDT, isOutput=False)
    y = nc.declare_dram_parameter("y", [T, D], F32, isOutput=True)

    with tile.TileContext(nc) as tc, ExitStack() as ctx:
        consts = ctx.enter_context(tc.tile_pool(name="consts", bufs=1))
        qkv = ctx.enter_context(tc.tile_pool(name="qkv", bufs=1))
        attn = ctx.enter_context(tc.tile_pool(name="attn", bufs=1))
        etp = ctx.enter_context(tc.tile_pool(name="etp", bufs=4))
        nrm = ctx.enter_context(tc.tile_pool(name="nrm", bufs=3))
        yevac = ctx.enter_context(tc.tile_pool(name="yevac", bufs=3))
        mmps = ctx.enter_context(tc.tile_pool(name="mmps", bufs=2, space="PSUM"))
        aps = ctx.enter_context(tc.tile_pool(name="aps", bufs=1, space="PSUM"))

        # ---- inputs / constants into SBUF
        x_sb = consts.tile([P, KT, T], MMDT)
        nc.sync.dma_start(
            out=x_sb, in_=xT[0:D, :].rearrange("(kt p) t -> p kt t", p=P)
        )
        ones_t = consts.tile([1, T], MMDT)
        nc.sync.dma_start(out=ones_t, in_=xT[D : D + 1, :])
        wq_sb = consts.tile([P, KT, 2 * P], MMDT)
        nc.sync.dma_start(
            out=wq_sb, in_=wqa.rearrange("(kt p) m -> p kt m", p=P)
        )

        wk_sb = consts.tile([P, KT, 2 * P], MMDT)
        nc.sync.dma_start(
            out=wk_sb, in_=wka.rearrange("(kt p) m -> p kt m", p=P)
        )

        wv_sb = consts.tile([P, KT, HPC * VW], MMDT)
        nc.sync.dma_start(
            out=wv_sb, in_=wva[0:D, :].rearrange("(kt p) m -> p kt m", p=P)
        )
        wvb_sb = consts.tile([1, HPC * VW], MMDT)
        nc.sync.dma_start(out=wvb_sb, in_=wva[D : D + 1, :])
        wqkb_sb = consts.tile([P, 4], F32)
        nc.sync.dma_start(out=wqkb_sb, in_=wqkb[:])
        wo_sb = consts.tile([HD, HPC, D], MMDT)
        nc.sync.dma_start(out=wo_sb, in_=wo.rearrange("(h p) d -> p h d", p=HD))

        # triu[k, q] = 1 iff q >= k: allowed region of a diagonal block in
        # transposed-score space.  gpsimd affine_select needs f32; cast after.
        triu_st = consts.tile([P, P], F32)
        make_upper_triangular(nc, triu_st, val=1.0, diag=True)
        triu = consts.tile([P, P], MMDT)
        nc.vector.tensor_copy(triu, triu_st)

        # ---- QKV projections (bias folded in via the ones row, K = D + 1)
        qT_sb = qkv.tile([P, 2, T], MMDT)
        kT_sb = qkv.tile([P, 2, T], MMDT)
        v_sb = qkv.tile([P, NKT, HPC * VW], MMDT)

        WB = 1024  # bf16 moving-operand max
        for wi, (w_sb, dst) in enumerate(((wq_sb, qT_sb), (wk_sb, kT_sb))):
            for m in range(2):
                for nb in range(T // WB):
                    ps = mmps.tile([P, WB], F32, tag="mm", name="ps")
                    for lo in range(0, WB, QB):  # psum-bank-sized writes
                        for kt in range(KT):
                            nc.tensor.matmul(
                                ps[:, lo : lo + QB],
                                lhsT=w_sb[:, kt, m * P : (m + 1) * P],
                                rhs=x_sb[:, kt, nb * WB + lo : nb * WB + lo + QB],
                                start=(kt == 0),
                                stop=(kt == KT - 1),
                            )
                    nc.scalar.activation(
                        out=dst[:, m, nb * WB : (nb + 1) * WB], in_=ps,
                        func=mybir.ActivationFunctionType.Identity,
                        bias=wqkb_sb[:, 2 * wi + m : 2 * wi + m + 1],
                    )

        for tt in range(NKT):
            ps = mmps.tile([P, QB], F32, tag="mm", name="ps")
            for kt in range(KT):
                nc.tensor.matmul(
                    ps[:, 0 : HPC * VW],
                    lhsT=x_sb[:, kt, tt * P : (tt + 1) * P],
                    rhs=wv_sb[:, kt, :],
                    start=(kt == 0),
                    stop=False,
                )
            nc.tensor.matmul(
                ps[:, 0 : HPC * VW],
                lhsT=ones_t[:, tt * P : (tt + 1) * P],
                rhs=wvb_sb,
                start=False,
                stop=True,
            )
            nc.vector.tensor_copy(v_sb[:, tt, :], ps[:, 0 : HPC * VW])

        # ---- attention, per head
        attn_h = [
            attn.tile([HD, T], MMDT, tag=f"attn{h}", name=f"attn{h}")
            for h in range(HPC)
        ]

        AB = 1024  # attention query-block width (bf16 moving max)
        NAB = T // AB
        for hp in range(HPC // 2):
            pair = (2 * hp, 2 * hp + 1)
            qhs = [qT_sb[(h % 2) * HD : (h % 2) * HD + HD, h // 2, :] for h in pair]
            khs = [kT_sb[(h % 2) * HD : (h % 2) * HD + HD, h // 2, :] for h in pair]
            for qb in range(NAB):
                opss = [
                    aps.tile([VW, AB], F32, tag=f"ops{i}", name=f"ops{i}")
                    for i in range(2)
                ]
                nkt = (qb + 1) * (AB // P)
                half0_last = min(nkt, qb * (AB // P) + QB // P) - 1
                for kt in range(nkt):
                    off = max(0, kt * P - qb * AB)
                    halves = [
                        (max(off, lo), lo + QB, lo)
                        for lo in range(0, AB, QB)
                        if off < lo + QB
                    ]
                    spss = []
                    for i in (0, 1):  # score matmuls: K=64 row-packed pair
                        sps = mmps.tile([P, AB], F32, tag="mm", name="sps")
                        for o, hi, _lo in halves:
                            nc.tensor.matmul(
                                sps[:, o:hi],
                                lhsT=khs[i][:, kt * P : (kt + 1) * P],
                                rhs=qhs[i][:, qb * AB + o : qb * AB + hi],
                                start=True,
                                stop=True,
                            )
                        spss.append(sps)
                    for i in (0, 1):
                        eT = etp.tile([P, AB], MMDT, tag="eT", name="eT")
                        nc.scalar.activation(
                            out=eT[:, off:AB], in_=spss[i][:, off:AB], func=EXP,
                            scale=1.0 / np.sqrt(HD),
                        )
                        if off > 0 or kt * P == qb * AB:  # diagonal-crossing
                            nc.vector.tensor_mul(
                                eT[:, off : off + P], eT[:, off : off + P], triu
                            )
                        for o, hi, lo in halves:
                            nc.tensor.matmul(
                                opss[i][:, o:hi],
                                lhsT=v_sb[:, kt, pair[i] * VW : (pair[i] + 1) * VW],
                                rhs=eT[:, o:hi],
                                start=(kt == 0),
                                stop=(kt == (half0_last if lo == 0 else nkt - 1)),
                            )
                # normalize: row HD of ops is the softmax denominator.
                # (engine ops keep base partition 0: full-tile copy + recip)
                for i in (0, 1):
                    att_sb = nrm.tile([VW, AB], F32, tag="att", name="att_sb")
                    nc.vector.tensor_copy(att_sb, opss[i])
                    rec = nrm.tile([VW, AB], F32, tag="rec", name="rec")
                    nc.vector.reciprocal_approx_fast(out=rec, in_=att_sb)
                    # broadcast the denominator-reciprocal row across the 64
                    # output partitions with a SWDGE copy (partition step 0)
                    # move the reciprocal row to partition 0 (DMA moves
                    # partitions freely; partition_broadcast reads abs row 0)
                    den0 = nrm.tile([1, AB], F32, tag="den0", name="den0")
                    nc.sync.dma_start(out=den0, in_=rec[HD : HD + 1, :])
                    bc = nrm.tile([HD, AB], F32, tag="bc", name="bc")
                    nc.gpsimd.partition_broadcast(bc, den0)
                    nc.vector.tensor_mul(
                        attn_h[pair[i]][:, qb * AB : (qb + 1) * AB],
                        att_sb[0:HD, :],
                        bc,
                    )

        # ---- out projection: y[t, d] = sum_h attn_h.T @ wo_h
        for tt in range(NKT):
            ps = mmps.tile([P, D], F32, tag="mm", name="ps")
            for h in range(HPC):
                nc.tensor.matmul(
                    ps,
                    lhsT=attn_h[h][:, tt * P : (tt + 1) * P],
                    rhs=wo_sb[:, h, :],
                    start=(h == 0),
                    stop=(h == HPC - 1),
                )
            yt = yevac.tile([P, D], F32, tag="yt", name="yt")
            nc.vector.tensor_copy(yt, ps)
            nc.sync.dma_start(out=y[tt * P : (tt + 1) * P, :], in_=yt)

    nc.compile()
    return nc


def make_in_maps(x, W_qkv, b_qkv, W_out):
    x = np.asarray(x, np.float32)
    W_qkv = np.asarray(W_qkv, np.float32)
    b_qkv = np.asarray(b_qkv, np.float32)
    W_out = np.asarray(W_out, np.float32)
    in_maps = []
    for c in range(2 * B):
        b, g = divmod(c, 2)
        ch = g * HPC * HD
        xTa = np.empty((D + 1, T), np.float32)
        xTa[:D] = x[b].T
        xTa[D] = 1.0
        wqa = W_qkv[:, ch : ch + 256]
        wka = W_qkv[:, D + ch : D + ch + 256]
        wqkb = np.concatenate(
            [
                b_qkv[ch : ch + 256].reshape(2, P).T,
                b_qkv[D + ch : D + ch + 256].reshape(2, P).T,
            ],
            axis=1,
        )  # [128, 4]: cols = q-m0, q-m1, k-m0, k-m1
        wva = np.zeros((D + 1, HPC * VW), np.float32)
        wva3 = wva[:D].reshape(D, HPC, VW)
        wva3[:, :, :HD] = W_qkv[:, 2 * D + ch : 2 * D + ch + 256].reshape(D, HPC, HD)
        wvab = wva[D].reshape(HPC, VW)
        wvab[:, :HD] = b_qkv[2 * D + ch : 2 * D + ch + 256].reshape(HPC, HD)
        wvab[:, HD] = 1.0
        in_maps.append(
            {
                "xT": xTa.astype(_NP_MMDT),
                "wqa": np.ascontiguousarray(wqa).astype(_NP_MMDT),
                "wka": np.ascontiguousarray(wka).astype(_NP_MMDT),
                "wva": wva.astype(_NP_MMDT),
                "wqkb": np.ascontiguousarray(wqkb, np.float32),
                "wo": np.ascontiguousarray(W_out[ch : ch + 256, :]).astype(_NP_MMDT),
            }
        )
    return in_maps


def assemble(results, b_out):
    b_out = np.asarray(b_out, np.float32)
    out = np.empty((B, T, D), np.float32)
    for b in range(B):
        out[b] = results[2 * b]["y"] + results[2 * b + 1]["y"] + b_out[None, :]
    return out


_CACHE = {}


def kernel(x, W_qkv, b_qkv, W_out, b_out):
    if "nc" not in _CACHE:
        _CACHE["nc"] = build_bass()
    in_maps = make_in_maps(x, W_qkv, b_qkv, W_out)
    res = run_bass_kernel_spmd(_CACHE["nc"], in_maps, list(range(2 * B)))
    return assemble(res.results, b_out)


# revision 17
# speedup vs baseline: 1.9735x; 1.0106x over previous
"""Multi-head causal self-attention on 8 Trainium2 NeuronCores.

Sharding: core c -> batch b = c // 2, heads 4*(c % 2) .. +4  (data parallel on
B, tensor parallel on heads).  Each core computes its 4 heads' attention for
its batch plus the partial out-projection; the host sums the two partials per
batch and adds b_out.

Per-core layout (engine-op APs stay at base partition 0; only PE operand
reads use base-64 slices):
  xT   [D+1, T]    x[b] transposed on host + ones row (folds b_qkv in as K+1)
  qT/kT [128, 2, T] head-major: partitions = 2 heads x 64, 2 m-tiles
  v    [128, 16, 260] natural [T, hd] per head + a ones column (gives the
                    softmax denominator for free during the AV matmul)
  scores are computed transposed: sT[k, q] = kT.T @ q, exp'd on ACT during
  PSUM->SBUF evacuation (no max subtraction: |scores| <= ~3 here), causality
  via tile skipping/trimming + one upper-triangular 128x128 mask constant.

Matmuls run in float32r (1 row/cycle vs 4 for float32, ~1.6e-4 rel err).
"""

import os
import sys
from contextlib import ExitStack

import numpy as np

for _p in ("/opt/trn_rl_repo", "/opt/pypackages"):
    if os.path.isdir(_p) and _p not in sys.path:
        sys.path.append(_p)

import concourse.bass as bass
from concourse import bacc
import concourse.mybir as mybir
import concourse.tile as tile
from concourse.bass_utils import run_bass_kernel_spmd
from concourse.masks import make_upper_triangular


B, T, D = 4, 2048, 512
H, HD = 8, 64
HPC = 4  # heads per core
P = 128
KT = D // P  # k-tiles over the model dim
QB = 512  # query block (free dim per matmul)
NQB = T // QB
NKT = T // P  # key tiles
VW = HD + 1  # v columns per head incl. the ones column

F32 = mybir.dt.float32
R32 = mybir.dt.float32r
BF16 = mybir.dt.bfloat16
# Matmul compute dtype for everything except the normalization broadcast
# (which stays f32r so the softmax denominators keep full precision).
MMDT = BF16 if os.environ.get("MHSA_DTYPE", "bf16") == "bf16" else R32
EXP = mybir.ActivationFunctionType.Exp

try:
    import ml_dtypes
    _NP_MMDT = np.float32 if MMDT == R32 else np.dtype(ml_dtypes.bfloat16)
except ImportError:
    _NP_MMDT = np.float32


def build_bass():
    nc = bacc.Bacc()
    xT = nc.declare_dram_parameter("xT", [D + 1, T], MMDT, isOutput=False)
    wqa = nc.declare_dram_parameter("wqa", [D, 2 * P], MMDT, isOutput=False)
    wka = nc.declare_dram_parameter("wka", [D, 2 * P], MMDT, isOutput=False)
    # q/k biases, laid out [channel % 128, channel // 128] for ACT bias APs
    wqkb = nc.declare_dram_parameter("wqkb", [P, 4], F32, isOutput=False)
    wva = nc.declare_dram_parameter("wva", [D + 1, HPC * VW], MMDT, isOutput=False)
    wo = nc.declare_dram_parameter("wo", [HPC * HD, D], MMDT, isOutput=False)
    y = nc.declare_dram_parameter("y", [T, D], F32, isOutput=True)

    with tile.TileContext(nc) as tc, ExitStack() as ctx:
        consts = ctx.enter_context(tc.tile_pool(name="consts", bufs=1))
        qkv = ctx.enter_context(tc.tile_pool(name="qkv", bufs=1))
        attn = ctx.enter_context(tc.tile_pool(name="attn", bufs=1))
        etp = ctx.enter_context(tc.tile_pool(name="etp", bufs=4))
        nrm = ctx.enter_context(tc.tile_pool(name="nrm", bufs=3))
        yevac = ctx.enter_context(tc.tile_pool(name="yevac", bufs=3))
        mmps = ctx.enter_context(tc.tile_pool(name="mmps", bufs=2, space="PSUM"))
        aps = ctx.enter_context(tc.tile_pool(name="aps", bufs=1, space="PSUM"))

        # ---- inputs / constants into SBUF
        _salt = consts.tile([1, 8], F32, name="salt")
        nc.vector.memset(_salt, float(os.environ.get("MHSA_SALT", "4")))
        x_sb = consts.tile([P, KT, T], MMDT)
        for kt in range(KT):
            nc.sync.dma_start(
                out=x_sb[:, kt, :], in_=xT[kt * P : (kt + 1) * P, :]
            )
        ones_t = consts.tile([1, T], MMDT)
        nc.sync.dma_start(out=ones_t, in_=xT[D : D + 1, :])
        wq_sb = consts.tile([P, KT, 2 * P], MMDT)
        nc.sync.dma_start(
            out=wq_sb, in_=wqa.rearrange("(kt p) m -> p kt m", p=P)
        )

        wk_sb = consts.tile([P, KT, 2 * P], MMDT)
        nc.sync.dma_start(
            out=wk_sb, in_=wka.rearrange("(kt p) m -> p kt m", p=P)
        )

        wv_sb = consts.tile([P, KT, HPC * VW], MMDT)
        nc.sync.dma_start(
            out=wv_sb, in_=wva[0:D, :].rearrange("(kt p) m -> p kt m", p=P)
        )
        wvb_sb = consts.tile([1, HPC * VW], MMDT)
        nc.sync.dma_start(out=wvb_sb, in_=wva[D : D + 1, :])
        wqkb_sb = consts.tile([P, 4], F32)
        nc.sync.dma_start(out=wqkb_sb, in_=wqkb[:])
        wo_sb = consts.tile([HD, HPC, D], MMDT)
        nc.sync.dma_start(out=wo_sb, in_=wo.rearrange("(h p) d -> p h d", p=HD))

        # triu[k, q] = 1 iff q >= k: allowed region of a diagonal block in
        # transposed-score space.  gpsimd affine_select needs f32; cast after.
        triu_st = consts.tile([P, P], F32)
        make_upper_triangular(nc, triu_st, val=1.0, diag=True)
        triu = consts.tile([P, P], MMDT)
        nc.vector.tensor_copy(triu, triu_st)

        # ---- QKV projections (bias folded in via the ones row, K = D + 1)
        qT_sb = qkv.tile([P, 2, T], MMDT)
        kT_sb = qkv.tile([P, 2, T], MMDT)
        v_sb = qkv.tile([P, NKT, HPC * VW], MMDT)

        WB = 1024  # bf16 moving-operand max
        for wi, (w_sb, dst) in enumerate(((wq_sb, qT_sb), (wk_sb, kT_sb))):
            for m in range(2):
                for nb in range(T // WB):
                    ps = mmps.tile([P, WB], F32, tag="mm", name="ps")
                    for lo in range(0, WB, QB):  # psum-bank-sized writes
                        for kt in range(KT):
                            nc.tensor.matmul(
                                ps[:, lo : lo + QB],
                                lhsT=w_sb[:, kt, m * P : (m + 1) * P],
                                rhs=x_sb[:, kt, nb * WB + lo : nb * WB + lo + QB],
                                start=(kt == 0),
                                stop=(kt == KT - 1),
                            )
                    nc.scalar.activation(
                        out=dst[:, m, nb * WB : (nb + 1) * WB], in_=ps,
                        func=mybir.ActivationFunctionType.Identity,
                        bias=wqkb_sb[:, 2 * wi + m : 2 * wi + m + 1],
                    )

        for tt in range(NKT):
            ps = mmps.tile([P, QB], F32, tag="mm", name="ps")
            for kt in range(KT):
                nc.tensor.matmul(
                    ps[:, 0 : HPC * VW],
                    lhsT=x_sb[:, kt, tt * P : (tt + 1) * P],
                    rhs=wv_sb[:, kt, :],
                    start=(kt == 0),
                    stop=False,
                )
            nc.tensor.matmul(
                ps[:, 0 : HPC * VW],
                lhsT=ones_t[:, tt * P : (tt + 1) * P],
                rhs=wvb_sb,
                start=False,
                stop=True,
            )
            nc.vector.tensor_copy(v_sb[:, tt, :], ps[:, 0 : HPC * VW])

        # ---- attention, per head
        attn_h = [
            attn.tile([HD, T], MMDT, tag=f"attn{h}", name=f"attn{h}")
            for h in range(HPC)
        ]

        AB = 1024  # attention query-block width (bf16 moving max)
        NAB = T // AB
        for hp in range(HPC // 2):
            pair = (2 * hp, 2 * hp + 1)
            qhs = [qT_sb[(h % 2) * HD : (h % 2) * HD + HD, h // 2, :] for h in pair]
            khs = [kT_sb[(h % 2) * HD : (h % 2) * HD + HD, h // 2, :] for h in pair]
            for qb in range(NAB):
                opss = [
                    aps.tile([VW, AB], F32, tag=f"ops{i}", name=f"ops{i}")
                    for i in range(2)
                ]
                nkt = (qb + 1) * (AB // P)
                half0_last = min(nkt, qb * (AB // P) + QB // P) - 1
                for kt in range(nkt):
                    off = max(0, kt * P - qb * AB)
                    halves = [
                        (max(off, lo), lo + QB, lo)
                        for lo in range(0, AB, QB)
                        if off < lo + QB
                    ]
                    spss = []
                    for i in (0, 1):  # score matmuls: K=64 row-packed pair
                        sps = mmps.tile([P, AB], F32, tag="mm", name="sps")
                        for o, hi, _lo in halves:
                            nc.tensor.matmul(
                                sps[:, o:hi],
                                lhsT=khs[i][:, kt * P : (kt + 1) * P],
                                rhs=qhs[i][:, qb * AB + o : qb * AB + hi],
                                start=True,
                                stop=True,
                            )
                        spss.append(sps)
                    for i in (0, 1):
                        eT = etp.tile([P, AB], MMDT, tag="eT", name="eT")
                        nc.scalar.activation(
                            out=eT[:, off:AB], in_=spss[i][:, off:AB], func=EXP,
                            scale=1.0 / np.sqrt(HD),
                        )
                        if off > 0 or kt * P == qb * AB:  # diagonal-crossing
                            nc.vector.tensor_mul(
                                eT[:, off : off + P], eT[:, off : off + P], triu
                            )
                        for o, hi, lo in halves:
                            nc.tensor.matmul(
                                opss[i][:, o:hi],
                                lhsT=v_sb[:, kt, pair[i] * VW : (pair[i] + 1) * VW],
                                rhs=eT[:, o:hi],
                                start=(kt == 0),
                                stop=(kt == (half0_last if lo == 0 else nkt - 1)),
                            )
                # normalize: row HD of ops is the softmax denominator.
                # (engine ops keep base partition 0: full-tile copy + recip)
                for i in (0, 1):
                    att_sb = nrm.tile([VW, AB], F32, tag="att", name="att_sb")
                    nc.vector.tensor_copy(att_sb, opss[i])
                    rec = nrm.tile([VW, AB], F32, tag="rec", name="rec")
                    nc.vector.reciprocal_approx_fast(out=rec, in_=att_sb)
                    # broadcast the denominator-reciprocal row across the 64
                    # output partitions with a SWDGE copy (partition step 0)
                    # move the reciprocal row to partition 0 (DMA moves
                    # partitions freely; partition_broadcast reads abs row 0)
                    den0 = nrm.tile([1, AB], F32, tag="den0", name="den0")
                    nc.sync.dma_start(out=den0, in_=rec[HD : HD + 1, :])
                    bc = nrm.tile([HD, AB], F32, tag="bc", name="bc")
                    nc.gpsimd.partition_broadcast(bc, den0)
                    nc.vector.tensor_mul(
                        attn_h[pair[i]][:, qb * AB : (qb + 1) * AB],
                        att_sb[0:HD, :],
                        bc,
                    )

        # ---- out projection: y[t, d] = sum_h attn_h.T @ wo_h
        for tt in range(NKT):
            ps = mmps.tile([P, D], F32, tag="mm", name="ps")
            for h in range(HPC):
                nc.tensor.matmul(
                    ps,
                    lhsT=attn_h[h][:, tt * P : (tt + 1) * P],
                    rhs=wo_sb[:, h, :],
                    start=(h == 0),
                    stop=(h == HPC - 1),
                )
            yt = yevac.tile([P, D], F32, tag="yt", name="yt")
            nc.vector.tensor_copy(yt, ps)
            nc.sync.dma_start(out=y[tt * P : (tt + 1) * P, :], in_=yt)

    nc.compile()
    return nc


def make_in_maps(x, W_qkv, b_qkv, W_out):
    x = np.asarray(x, np.float32)
    W_qkv = np.asarray(W_qkv, np.float32)
    b_qkv = np.asarray(b_qkv, np.float32)
    W_out = np.asarray(W_out, np.float32)
    in_maps = []
    for c in range(2 * B):
        b, g = divmod(c, 2)
        ch = g * HPC * HD
        xTa = np.empty((D + 1, T), np.float32)
        xTa[:D] = x[b].T
        xTa[D] = 1.0
        wqa = W_qkv[:, ch : ch + 256]
        wka = W_qkv[:, D + ch : D + ch + 256]
        wqkb = np.concatenate(
            [
                b_qkv[ch : ch + 256].reshape(2, P).T,
                b_qkv[D + ch : D + ch + 256].reshape(2, P).T,
            ],
            axis=1,
        )  # [128, 4]: cols = q-m0, q-m1, k-m0, k-m1
        wva = np.zeros((D + 1, HPC * VW), np.float32)
        wva3 = wva[:D].reshape(D, HPC, VW)
        wva3[:, :, :HD] = W_qkv[:, 2 * D + ch : 2 * D + ch + 256].reshape(D, HPC, HD)
        wvab = wva[D].reshape(HPC, VW)
        wvab[:, :HD] = b_qkv[2 * D + ch : 2 * D + ch + 256].reshape(HPC, HD)
        wvab[:, HD] = 1.0
        in_maps.append(
            {
                "xT": xTa.astype(_NP_MMDT),
                "wqa": np.ascontiguousarray(wqa).astype(_NP_MMDT),
                "wka": np.ascontiguousarray(wka).astype(_NP_MMDT),
                "wva": wva.astype(_NP_MMDT),
                "wqkb": np.ascontiguousarray(wqkb, np.float32),
                "wo": np.ascontiguousarray(W_out[ch : ch + 256, :]).astype(_NP_MMDT),
            }
        )
    return in_maps


def assemble(results, b_out):
    b_out = np.asarray(b_out, np.float32)
    out = np.empty((B, T, D), np.float32)
    for b in range(B):
        out[b] = results[2 * b]["y"] + results[2 * b + 1]["y"] + b_out[None, :]
    return out


_CACHE = {}


def kernel(x, W_qkv, b_qkv, W_out, b_out):
    if "nc" not in _CACHE:
        _CACHE["nc"] = build_bass()
    in_maps = make_in_maps(x, W_qkv, b_qkv, W_out)
    res = run_bass_kernel_spmd(_CACHE["nc"], in_maps, list(range(2 * B)))
    return assemble(res.results, b_out)


# revision 18
# speedup vs baseline: 2.0061x; 1.0165x over previous
"""Multi-head causal self-attention on 8 Trainium2 NeuronCores.

Sharding: core c -> batch b = c // 2, heads 4*(c % 2) .. +4  (data parallel on
B, tensor parallel on heads).  Each core computes its 4 heads' attention for
its batch plus the partial out-projection; the host sums the two partials per
batch and adds b_out.

Per-core layout (engine-op APs stay at base partition 0; only PE operand
reads use base-64 slices):
  xT   [D+1, T]    x[b] transposed on host + ones row (folds b_qkv in as K+1)
  qT/kT [128, 2, T] head-major: partitions = 2 heads x 64, 2 m-tiles
  v    [128, 16, 260] natural [T, hd] per head + a ones column (gives the
                    softmax denominator for free during the AV matmul)
  scores are computed transposed: sT[k, q] = kT.T @ q, exp'd on ACT during
  PSUM->SBUF evacuation (no max subtraction: |scores| <= ~3 here), causality
  via tile skipping/trimming + one upper-triangular 128x128 mask constant.

Matmuls run in float32r (1 row/cycle vs 4 for float32, ~1.6e-4 rel err).
"""

import os
import sys
from contextlib import ExitStack

import numpy as np

for _p in ("/opt/trn_rl_repo", "/opt/pypackages"):
    if os.path.isdir(_p) and _p not in sys.path:
        sys.path.append(_p)

import concourse.bass as bass
from concourse import bacc
import concourse.mybir as mybir
import concourse.tile as tile
from concourse.bass_utils import run_bass_kernel_spmd
from concourse.masks import make_upper_triangular


B, T, D = 4, 2048, 512
H, HD = 8, 64
HPC = 4  # heads per core
P = 128
KT = D // P  # k-tiles over the model dim
QB = 512  # query block (free dim per matmul)
NQB = T // QB
NKT = T // P  # key tiles
VW = HD + 1  # v columns per head incl. the ones column

F32 = mybir.dt.float32
R32 = mybir.dt.float32r
BF16 = mybir.dt.bfloat16
# Matmul compute dtype for everything except the normalization broadcast
# (which stays f32r so the softmax denominators keep full precision).
MMDT = BF16 if os.environ.get("MHSA_DTYPE", "bf16") == "bf16" else R32
EXP = mybir.ActivationFunctionType.Exp

try:
    import ml_dtypes
    _NP_MMDT = np.float32 if MMDT == R32 else np.dtype(ml_dtypes.bfloat16)
except ImportError:
    _NP_MMDT = np.float32


def build_bass():
    nc = bacc.Bacc()
    xT = nc.declare_dram_parameter("xT", [D + 1, T], MMDT, isOutput=False)
    wqa = nc.declare_dram_parameter("wqa", [D, 2 * P], MMDT, isOutput=False)
    wka = nc.declare_dram_parameter("wka", [D, 2 * P], MMDT, isOutput=False)
    # q/k biases, laid out [channel % 128, channel // 128] for ACT bias APs
    wqkb = nc.declare_dram_parameter("wqkb", [P, 4], F32, isOutput=False)
    wva = nc.declare_dram_parameter("wva", [D, HPC * VW], MMDT, isOutput=False)
    wo = nc.declare_dram_parameter("wo", [HPC * HD, D], MMDT, isOutput=False)
    y = nc.declare_dram_parameter("y", [T, D], F32, isOutput=True)

    with tile.TileContext(nc) as tc, ExitStack() as ctx:
        consts = ctx.enter_context(tc.tile_pool(name="consts", bufs=1))
        qkv = ctx.enter_context(tc.tile_pool(name="qkv", bufs=1))
        attn = ctx.enter_context(tc.tile_pool(name="attn", bufs=1))
        etp = ctx.enter_context(tc.tile_pool(name="etp", bufs=4))
        nrm = ctx.enter_context(tc.tile_pool(name="nrm", bufs=3))
        yevac = ctx.enter_context(tc.tile_pool(name="yevac", bufs=3))
        mmps = ctx.enter_context(tc.tile_pool(name="mmps", bufs=2, space="PSUM"))
        aps = ctx.enter_context(tc.tile_pool(name="aps", bufs=1, space="PSUM"))

        # ---- inputs / constants into SBUF
        _salt = consts.tile([1, 8], F32, name="salt")
        nc.vector.memset(_salt, float(os.environ.get("MHSA_SALT", "4")))
        x_sb = consts.tile([P, KT, T], MMDT)
        for kt in range(KT):
            nc.sync.dma_start(
                out=x_sb[:, kt, :], in_=xT[kt * P : (kt + 1) * P, :]
            )
        wq_sb = consts.tile([P, KT, 2 * P], MMDT)
        nc.sync.dma_start(
            out=wq_sb, in_=wqa.rearrange("(kt p) m -> p kt m", p=P)
        )

        wk_sb = consts.tile([P, KT, 2 * P], MMDT)
        nc.sync.dma_start(
            out=wk_sb, in_=wka.rearrange("(kt p) m -> p kt m", p=P)
        )

        wv_sb = consts.tile([P, KT, HPC * VW], MMDT)
        nc.sync.dma_start(
            out=wv_sb, in_=wva.rearrange("(kt p) m -> p kt m", p=P)
        )
        wqkb_sb = consts.tile([P, 4], F32)
        nc.sync.dma_start(out=wqkb_sb, in_=wqkb[:])
        wo_sb = consts.tile([HD, HPC, D], MMDT)
        nc.sync.dma_start(out=wo_sb, in_=wo.rearrange("(h p) d -> p h d", p=HD))

        # triu[k, q] = 1 iff q >= k: allowed region of a diagonal block in
        # transposed-score space.  gpsimd affine_select needs f32; cast after.
        triu_st = consts.tile([P, P], F32)
        make_upper_triangular(nc, triu_st, val=1.0, diag=True)
        triu = consts.tile([P, P], MMDT)
        nc.vector.tensor_copy(triu, triu_st)

        # ---- QKV projections (bias folded in via the ones row, K = D + 1)
        qT_sb = qkv.tile([P, 2, T], MMDT)
        kT_sb = qkv.tile([P, 2, T], MMDT)
        v_sb = qkv.tile([P, NKT, HPC * VW], MMDT)

        WB = 1024  # bf16 moving-operand max
        for wi, (w_sb, dst) in enumerate(((wq_sb, qT_sb), (wk_sb, kT_sb))):
            for m in range(2):
                for nb in range(T // WB):
                    ps = mmps.tile([P, WB], F32, tag="mm", name="ps")
                    for lo in range(0, WB, QB):  # psum-bank-sized writes
                        for kt in range(KT):
                            nc.tensor.matmul(
                                ps[:, lo : lo + QB],
                                lhsT=w_sb[:, kt, m * P : (m + 1) * P],
                                rhs=x_sb[:, kt, nb * WB + lo : nb * WB + lo + QB],
                                start=(kt == 0),
                                stop=(kt == KT - 1),
                            )
                    nc.scalar.activation(
                        out=dst[:, m, nb * WB : (nb + 1) * WB], in_=ps,
                        func=mybir.ActivationFunctionType.Identity,
                        bias=wqkb_sb[:, 2 * wi + m : 2 * wi + m + 1],
                    )

        # v bias is folded into the host-side output bias (b_v @ W_out adds a
        # constant row after softmax-normalize + out-projection), so v here is
        # bias-free; the denominator ones-columns are memset directly.
        for tt in range(NKT):
            ps = mmps.tile([P, QB], F32, tag="mm", name="ps")
            for kt in range(KT):
                nc.tensor.matmul(
                    ps[:, 0 : HPC * VW],
                    lhsT=x_sb[:, kt, tt * P : (tt + 1) * P],
                    rhs=wv_sb[:, kt, :],
                    start=(kt == 0),
                    stop=(kt == KT - 1),
                )
            nc.vector.tensor_copy(v_sb[:, tt, :], ps[:, 0 : HPC * VW])
            ones_cols = v_sb[:, tt, :].rearrange("p (h w) -> p h w", w=VW)[:, :, HD]
            nc.vector.memset(ones_cols, 1.0)

        # ---- attention, per head
        attn_h = [
            attn.tile([HD, T], MMDT, tag=f"attn{h}", name=f"attn{h}")
            for h in range(HPC)
        ]

        AB = 1024  # attention query-block width (bf16 moving max)
        NAB = T // AB
        for hp in range(HPC // 2):
            pair = (2 * hp, 2 * hp + 1)
            qhs = [qT_sb[(h % 2) * HD : (h % 2) * HD + HD, h // 2, :] for h in pair]
            khs = [kT_sb[(h % 2) * HD : (h % 2) * HD + HD, h // 2, :] for h in pair]
            for qb in range(NAB):
                opss = [
                    aps.tile([VW, AB], F32, tag=f"ops{i}", name=f"ops{i}")
                    for i in range(2)
                ]
                nkt = (qb + 1) * (AB // P)
                half0_last = min(nkt, qb * (AB // P) + QB // P) - 1
                for kt in range(nkt):
                    off = max(0, kt * P - qb * AB)
                    halves = [
                        (max(off, lo), lo + QB, lo)
                        for lo in range(0, AB, QB)
                        if off < lo + QB
                    ]
                    spss = []
                    for i in (0, 1):  # score matmuls: K=64 row-packed pair
                        sps = mmps.tile([P, AB], F32, tag="mm", name="sps")
                        for o, hi, _lo in halves:
                            nc.tensor.matmul(
                                sps[:, o:hi],
                                lhsT=khs[i][:, kt * P : (kt + 1) * P],
                                rhs=qhs[i][:, qb * AB + o : qb * AB + hi],
                                start=True,
                                stop=True,
                            )
                        spss.append(sps)
                    for i in (0, 1):
                        eT = etp.tile([P, AB], MMDT, tag="eT", name="eT")
                        nc.scalar.activation(
                            out=eT[:, off:AB], in_=spss[i][:, off:AB], func=EXP,
                            scale=1.0 / np.sqrt(HD),
                        )
                        if off > 0 or kt * P == qb * AB:  # diagonal-crossing
                            nc.vector.tensor_mul(
                                eT[:, off : off + P], eT[:, off : off + P], triu
                            )
                        for o, hi, lo in halves:
                            nc.tensor.matmul(
                                opss[i][:, o:hi],
                                lhsT=v_sb[:, kt, pair[i] * VW : (pair[i] + 1) * VW],
                                rhs=eT[:, o:hi],
                                start=(kt == 0),
                                stop=(kt == (half0_last if lo == 0 else nkt - 1)),
                            )
                # normalize: row HD of ops is the softmax denominator.
                # (engine ops keep base partition 0: full-tile copy + recip)
                for i in (0, 1):
                    att_sb = nrm.tile([VW, AB], F32, tag="att", name="att_sb")
                    nc.vector.tensor_copy(att_sb, opss[i])
                    rec = nrm.tile([VW, AB], F32, tag="rec", name="rec")
                    nc.vector.reciprocal_approx_fast(out=rec, in_=att_sb)
                    # broadcast the denominator-reciprocal row across the 64
                    # output partitions with a SWDGE copy (partition step 0)
                    # move the reciprocal row to partition 0 (DMA moves
                    # partitions freely; partition_broadcast reads abs row 0)
                    den0 = nrm.tile([1, AB], F32, tag="den0", name="den0")
                    nc.sync.dma_start(out=den0, in_=rec[HD : HD + 1, :])
                    bc = nrm.tile([HD, AB], F32, tag="bc", name="bc")
                    nc.gpsimd.partition_broadcast(bc, den0)
                    nc.vector.tensor_mul(
                        attn_h[pair[i]][:, qb * AB : (qb + 1) * AB],
                        att_sb[0:HD, :],
                        bc,
                    )

        # ---- out projection: y[t, d] = sum_h attn_h.T @ wo_h
        for tt in range(NKT):
            ps = mmps.tile([P, D], F32, tag="mm", name="ps")
            for h in range(HPC):
                nc.tensor.matmul(
                    ps,
                    lhsT=attn_h[h][:, tt * P : (tt + 1) * P],
                    rhs=wo_sb[:, h, :],
                    start=(h == 0),
                    stop=(h == HPC - 1),
                )
            yt = yevac.tile([P, D], F32, tag="yt", name="yt")
            nc.vector.tensor_copy(yt, ps)
            nc.sync.dma_start(out=y[tt * P : (tt + 1) * P, :], in_=yt)

    nc.compile()
    return nc


def make_in_maps(x, W_qkv, b_qkv, W_out):
    x = np.asarray(x, np.float32)
    W_qkv = np.asarray(W_qkv, np.float32)
    b_qkv = np.asarray(b_qkv, np.float32)
    W_out = np.asarray(W_out, np.float32)
    in_maps = []
    for c in range(2 * B):
        b, g = divmod(c, 2)
        ch = g * HPC * HD
        xTa = np.empty((D + 1, T), np.float32)
        xTa[:D] = x[b].T
        xTa[D] = 1.0
        wqa = W_qkv[:, ch : ch + 256]
        wka = W_qkv[:, D + ch : D + ch + 256]
        wqkb = np.concatenate(
            [
                b_qkv[ch : ch + 256].reshape(2, P).T,
                b_qkv[D + ch : D + ch + 256].reshape(2, P).T,
            ],
            axis=1,
        )  # [128, 4]: cols = q-m0, q-m1, k-m0, k-m1
        wva = np.zeros((D, HPC * VW), np.float32)
        wva3 = wva.reshape(D, HPC, VW)
        wva3[:, :, :HD] = W_qkv[:, 2 * D + ch : 2 * D + ch + 256].reshape(D, HPC, HD)
        in_maps.append(
            {
                "xT": xTa.astype(_NP_MMDT),
                "wqa": np.ascontiguousarray(wqa).astype(_NP_MMDT),
                "wka": np.ascontiguousarray(wka).astype(_NP_MMDT),
                "wva": wva.astype(_NP_MMDT),
                "wqkb": np.ascontiguousarray(wqkb, np.float32),
                "wo": np.ascontiguousarray(W_out[ch : ch + 256, :]).astype(_NP_MMDT),
            }
        )
    return in_maps


def assemble(results, b_out, vbias_y):
    b_out = np.asarray(b_out, np.float32) + vbias_y
    out = np.empty((B, T, D), np.float32)
    for b in range(B):
        out[b] = results[2 * b]["y"] + results[2 * b + 1]["y"] + b_out[None, :]
    return out


_CACHE = {}


def kernel(x, W_qkv, b_qkv, W_out, b_out):
    if "nc" not in _CACHE:
        _CACHE["nc"] = build_bass()
    in_maps = make_in_maps(x, W_qkv, b_qkv, W_out)
    # v-bias contribution: softmax weights sum to 1, so b_v passes through
    # attention unchanged and lands as (b_v @ W_out) on every token.
    vbias_y = np.asarray(b_qkv, np.float32)[2 * D :] @ np.asarray(W_out, np.float32)
    res = run_bass_kernel_spmd(_CACHE["nc"], in_maps, list(range(2 * B)))
    return assemble(res.results, b_out, vbias_y)
